# revision 7
# baseline (speedup 1.0000x reference)
"""ASSD (average symmetric surface distance) kernel for Trainium2, 8 NeuronCores.

Problem: real_pts [16384,3], pred_pts [16384,3] in [0,128)^3.
  assd = (sum_i NNdist(pred_i, real) + sum_j NNdist(real_j, pred)) / 32768

Strategy
--------
Host (cheap, O(N log N)): bin each query set into y-stripes, sort by z
inside each stripe, and cut into blocks of 128 queries. For each block,
gather the reference points whose (y, z) lie within MARGIN of the block's
bounding box into a fixed-width padded candidate window of W points.
A query's true nearest neighbor at distance d <= MARGIN is always inside
that window, so the windowed min equals the true min whenever the result
is <= MARGIN — which the host verifies per query (guard). If any query
fails the guard (can only happen for inputs much sparser than the target
workload), fall back to an exact brute-force evaluation, so the kernel
is correct for ANY input.

HW (the O(N*W) compute): per block, a K=5 augmented matmul computes the
full squared-distance matrix directly in PSUM, accumulating in the same
order as the reference (q2 + r2 - 2 q.r, fp32):
  lhsT rows: [q2, 1, -2qx, -2qy, -2qz],  rhs rows: [1, r2, rx, ry, rz]
then a DVE reduce_min over the window -> per-query min d2.
Host finishes: d = sqrt(max(d2, 0)), masked sum, divide.

The 8 cores each process an equal share of the (both-direction) block list.
"""

import numpy as np

BLK = 128          # queries per block (PE output partitions)
W = 768            # padded candidate window (1.5 PSUM banks)
S = 8              # y-stripes
MARGIN = 3.0       # NN-distance bound the windows guarantee
GUARD = MARGIN - 0.01
N_CORES = 8
GROUP = 2          # blocks per PSUM tile / per DVE reduce
BIG = 1.0e9        # pad candidate "r2" -> never the min

_nc_cache = {}
LAST_RESULT = None  # BassKernelResults of the last HW run (for profiling)


def _build_bass(nb, w):
    """Bass kernel: nb blocks of (q [5,128] x c [5,w]) matmul + reduce_min.

    nb must be a multiple of GROUP. Output o[lane, block] = min d2.
    """
    from concourse import mybir, tile, bacc

    f32 = mybir.dt.float32
    nc = bacc.Bacc()
    q_d = nc.declare_dram_parameter("q", [nb, 5, BLK], f32, isOutput=False)
    c_d = nc.declare_dram_parameter("c", [nb, 5, w], f32, isOutput=False)
    o_d = nc.declare_dram_parameter("o", [BLK, nb], f32, isOutput=True)

    with tile.TileContext(nc) as tc:
        with (
            tc.tile_pool(name="sb", bufs=3) as sb,
            tc.tile_pool(name="ps", bufs=2, space="PSUM") as pp,
            tc.tile_pool(name="accp", bufs=1) as apool,
        ):
            acc = apool.tile([BLK, nb], f32)
            for g in range(nb // GROUP):
                qt = sb.tile([5, GROUP * BLK], f32, tag="q")
                ct = sb.tile([5, GROUP * w], f32, tag="c")
                for j in range(GROUP):
                    nc.sync.dma_start(
                        qt[:, j * BLK:(j + 1) * BLK], q_d[g * GROUP + j]
                    )
                    nc.sync.dma_start(
                        ct[:, j * w:(j + 1) * w], c_d[g * GROUP + j]
                    )
                # PSUM slot per block padded to a bank multiple (1024) so
                # every matmul write stays inside one bank; the reduce
                # reads only the w valid columns via a strided AP.
                wpad = -(-w // 512) * 512
                ps = pp.tile([BLK, GROUP, wpad], f32)
                for j in range(GROUP):
                    for off in range(0, w, 512):
                        sz = min(512, w - off)
                        nc.tensor.matmul(
                            ps[:, j, off:off + sz],
                            qt[:, j * BLK:(j + 1) * BLK],
                            ct[:, j * w + off:j * w + off + sz],
                        )
                nc.vector.tensor_reduce(
                    acc[:, g * GROUP:(g + 1) * GROUP], ps[:, :, :w],
                    axis=mybir.AxisListType.X, op=mybir.AluOpType.min,
                )
            nc.sync.dma_start(o_d[:], acc[:])
    nc.compile()
    return nc


def _make_blocks(qpts, rpts):
    """Cut queries into y-stripe/z-sorted blocks; gather candidate windows.

    Returns (q_aug [nb,5,128], c_aug [nb,5,W], mask [nb,128], ok).
    ok=False when some window overflowed W (caller must fall back).
    """
    n = qpts.shape[0]
    stripe_h = 128.0 / S
    sid = np.minimum(qpts[:, 1] // stripe_h, S - 1).astype(np.int64)
    order = np.lexsort((qpts[:, 2], sid))
    qs = qpts[order]
    ss = sid[order]

    r2 = (rpts * rpts).sum(1, dtype=np.float32)
    q2s = (qs * qs).sum(1, dtype=np.float32)
    ry = rpts[:, 1]
    rz = rpts[:, 2]
    rorder = np.argsort(rz)
    rz_s = rz[rorder]

    q_blocks, c_blocks, m_blocks = [], [], []
    ok = True
    start = 0
    while start < n:
        s = ss[start]
        send = np.searchsorted(ss, s, side="right")
        bend = min(start + BLK, send)
        mem = qs[start:bend]
        cnt = mem.shape[0]

        zlo, zhi = mem[:, 2].min() - MARGIN, mem[:, 2].max() + MARGIN
        ylo, yhi = mem[:, 1].min() - MARGIN, mem[:, 1].max() + MARGIN
        i0 = np.searchsorted(rz_s, zlo, side="left")
        i1 = np.searchsorted(rz_s, zhi, side="right")
        cand_idx = rorder[i0:i1]
        cand_idx = cand_idx[(ry[cand_idx] >= ylo) & (ry[cand_idx] <= yhi)]
        ncand = cand_idx.shape[0]
        if ncand > W:
            ok = False
            break

        ca = np.empty((5, W), np.float32)
        ca[0, :] = 1.0
        ca[1, :ncand] = r2[cand_idx]
        ca[2, :ncand] = rpts[cand_idx, 0]
        ca[3, :ncand] = rpts[cand_idx, 1]
        ca[4, :ncand] = rpts[cand_idx, 2]
        ca[1, ncand:] = BIG
        ca[2:, ncand:] = 0.0

        qa = np.zeros((5, BLK), np.float32)
        qa[0, :cnt] = q2s[start:bend]
        qa[1, :] = 1.0
        qa[2, :cnt] = -2.0 * mem[:, 0]
        qa[3, :cnt] = -2.0 * mem[:, 1]
        qa[4, :cnt] = -2.0 * mem[:, 2]

        msk = np.zeros(BLK, bool)
        msk[:cnt] = True

        q_blocks.append(qa)
        c_blocks.append(ca)
        m_blocks.append(msk)
        start = bend

    if not ok:
        return None, None, None, False
    return np.stack(q_blocks), np.stack(c_blocks), np.stack(m_blocks), True


def _brute_force(real, pred):
    """Exact fallback, mirrors reference numerics in fp32 (blocked)."""
    def nn_sum(q, r):
        r2 = (r * r).sum(1, dtype=np.float32)[None, :]
        q2 = (q * q).sum(1, dtype=np.float32)[:, None]
        tot = 0.0
        for i in range(0, q.shape[0], 1024):
            d2 = q2[i:i + 1024] + r2 - np.float32(2.0) * (q[i:i + 1024] @ r.T)
            d2 = np.maximum(d2, 0.0)
            tot += np.sqrt(d2.min(1)).astype(np.float64).sum()
        return tot
    n = real.shape[0] + pred.shape[0]
    return (nn_sum(pred, real) + nn_sum(real, pred)) / n


def kernel(real_pts, pred_pts):
    global LAST_RESULT
    real = np.ascontiguousarray(np.asarray(real_pts, dtype=np.float32))
    pred = np.ascontiguousarray(np.asarray(pred_pts, dtype=np.float32))

    qa1, ca1, m1, ok1 = _make_blocks(pred, real)   # pred -> real
    qa2, ca2, m2, ok2 = _make_blocks(real, pred)   # real -> pred
    if not (ok1 and ok2):
        return np.float32(_brute_force(real, pred))

    qa = np.concatenate([qa1, qa2])
    ca = np.concatenate([ca1, ca2])
    msk = np.concatenate([m1, m2])

    total = qa.shape[0]
    per = N_CORES * GROUP
    nb = -(-total // per) * GROUP      # blocks per core, multiple of GROUP
    padded = nb * N_CORES
    if padded > total:
        npad = padded - total
        padq = np.zeros((npad, 5, BLK), np.float32)
        padq[:, 1, :] = 1.0
        padc = np.zeros((npad, 5, W), np.float32)
        padc[:, 0, :] = 1.0
        padc[:, 1, :] = BIG
        qa = np.concatenate([qa, padq])
        ca = np.concatenate([ca, padc])
        msk = np.concatenate([msk, np.zeros((npad, BLK), bool)])

    if nb not in _nc_cache:
        _nc_cache[nb] = _build_bass(nb, W)
    nc = _nc_cache[nb]

    from concourse.bass_utils import run_bass_kernel_spmd
    in_maps = [
        {"q": np.ascontiguousarray(qa[i * nb:(i + 1) * nb]),
         "c": np.ascontiguousarray(ca[i * nb:(i + 1) * nb])}
        for i in range(N_CORES)
    ]
    res = run_bass_kernel_spmd(nc, in_maps, list(range(N_CORES)))
    LAST_RESULT = res

    # o[core] is [128, nb]: lane l of block b -> min d2
    d2 = np.concatenate(
        [res.results[i]["o"].T for i in range(N_CORES)], axis=0
    )  # [padded, 128]
    d = np.sqrt(np.maximum(d2.astype(np.float64), 0.0))
    dv = d[msk]
    if dv.size != real.shape[0] + pred.shape[0] or (dv > GUARD).any():
        return np.float32(_brute_force(real, pred))
    assd = dv.sum() / (real.shape[0] + pred.shape[0])
    return np.float32(assd)
